# revision 21
# baseline (speedup 1.0000x reference)
# Multi-head attention (B=4, T=2048, D=1024, H=16, dqk=dv=64) on 8 trn2
# NeuronCores. Sharding: core c -> batch c//2, head-group c%2 (8 heads).
# Each core computes its batch's Q^T/K^T/V projections for its heads,
# causal flash attention with transposed scores (S^T[kv,q]; softmax
# normalizer via a ones-column appended to V), and a partial output
# projection. Host sums the two partials per batch and adds biases.
#
# Per-core structure: the attention inner loop is ACT(exp)-limited, so
# exp is batched across the head pair (one [128,2,512] ACTIVATE per kv
# tile), the padding mask is folded multiplicatively into V (uniform
# zero ACT bias), the causal mask multiply covers only the 128-col
# triangle band, the softmax-normalizer broadcast runs on the idle
# GPSIMD instead of the PE, and all non-attention matmul work
# (projections, output projection, finalize) is drained from a global
# micro-step queue at a fixed rate per attention iteration so the PE
# stream stays dense (HAM stays warm).
import numpy as np
import ml_dtypes

B, TQ, TKV, DM, H, DQ, DV = 4, 2048, 2048, 1024, 16, 64, 64
NC = 8          # cores
HL = 8          # heads per core
NHP = HL // 2   # 128-partition head-pair tiles (4)
SB = 512        # q super-block width
NQSB = TQ // SB
NKT = TKV // 128
NDM = DM // 128
P = 128

bf16 = ml_dtypes.bfloat16

_programs = {}
_last_in_maps = None


def _make_tc_class(tile_mod):
    from concourse.vector_clock import ScopedClock
    import concourse.mybir as mybir

    class TC(tile_mod.TileContext):
        # This toolchain's walrus codegen encodes at most ONE sync wait
        # per instruction. Tile's wait assignment can attach several, so
        # before lowering, peel extra waits off onto standalone
        # InstEventSemaphore instructions placed immediately before the
        # instruction on the same engine (in-order execution makes this
        # semantically identical).
        def _lower_ordered_insts(self, ordered):
            for bb_name, insts in ordered.items():
                out = []
                for inst in insts:
                    si = getattr(inst, "sync_info", None)
                    eng = getattr(inst, "engine", None)
                    if (
                        si is not None
                        and si.on_wait
                        and len(si.on_wait) > 1
                        and eng is not None
                        and eng != mybir.EngineType.Unassigned
                    ):
                        waits = list(si.on_wait)
                        for w in waits[:-1]:
                            ev = mybir.InstEventSemaphore(
                                name=f"I-{self.nc.next_id()}", ins=[], outs=[]
                            )
                            ev.engine = eng
                            ev.sync_info = mybir.SyncInfo(
                                on_wait=[w], on_update=[]
                            )
                            out.append(ev)
                        si.on_wait = waits[-1:]
                    out.append(inst)
                insts[:] = out
            return super()._lower_ordered_insts(ordered)

        # Same 1-wait limit applies to the tail drain; split its waits
        # into standalone wait instructions.
        def _drain_and_barrier(self, tick_clock, wait_clock):
            drain_inst = self.nc.sync.drain()
            wait_clock.add_sem_waits(
                drain_inst.ins, ScopedClock({None: tick_clock.global_clock})
            )
            si = drain_inst.ins.sync_info
            waits = list(si.on_wait) if si and si.on_wait else []
            if len(waits) > 1:
                si.on_wait = waits[:1]
                name2sem = {}
                for s in self.sems.allocated().values():
                    name2sem[getattr(s, "name", None) or str(s)] = s
                for w in waits[1:]:
                    self.nc.sync.wait_ge(name2sem[w.ant_name], w.wait_value)
            self.nc.all_engine_barrier()
            popped = self.nc._tile_sem_poison_stack.pop()
            assert popped is self._sem_poison
            self.nc.clear_and_free_semaphores(list(self.sems.allocated().values()))
            self.nc.all_engine_barrier()

    return TC


def build_program(causal: bool):
    import concourse.bass as bass
    import concourse.mybir as mybir
    import concourse.tile as tile

    dt = mybir.dt
    AF = mybir.ActivationFunctionType
    TC = _make_tc_class(tile)

    nc = bass.Bass("TRN2", target_bir_lowering=False, debug=False, num_devices=NC)

    xqT = nc.dram_tensor("xqT", [DM, TQ], dt.bfloat16, kind="ExternalInput")
    xkvT = nc.dram_tensor("xkvT", [DM, TKV], dt.bfloat16, kind="ExternalInput")
    wq_d = nc.dram_tensor("wq", [DM, HL * DQ], dt.bfloat16, kind="ExternalInput")
    wk_d = nc.dram_tensor("wk", [DM, HL * DQ], dt.bfloat16, kind="ExternalInput")
    wv_d = nc.dram_tensor("wv", [DM, HL * DV], dt.bfloat16, kind="ExternalInput")
    wo_d = nc.dram_tensor("wo", [HL * DV, DM], dt.bfloat16, kind="ExternalInput")
    bq_d = nc.dram_tensor("bqp", [P, NHP], dt.float32, kind="ExternalInput")
    bk_d = nc.dram_tensor("bkp", [P, NHP], dt.float32, kind="ExternalInput")
    padm_d = nc.dram_tensor("padm", [P, NKT], dt.float32, kind="ExternalInput")
    msk_d = nc.dram_tensor("msk2", [P, 2 * P], dt.bfloat16, kind="ExternalInput")
    one_d = nc.dram_tensor("one64", [P, HL * 64], dt.bfloat16, kind="ExternalInput")
    out_d = nc.dram_tensor("out", [TQ, DM], dt.float32, kind="ExternalOutput")

    with TC(nc) as tc:
        with (
            tc.tile_pool(name="res", bufs=1) as res,
            tc.tile_pool(name="xp", bufs=16) as xp,
            tc.tile_pool(name="ptp", bufs=4) as ptp,
            tc.tile_pool(name="atp", bufs=2) as atp,
            tc.tile_pool(name="anp", bufs=2) as anp,
            tc.tile_pool(name="rcp", bufs=2) as rcp,
            tc.tile_pool(name="osp", bufs=3) as osp,
            tc.tile_pool(name="ps_proj", bufs=2, space="PSUM") as ps_proj,
            tc.tile_pool(name="ps_s", bufs=2, space="PSUM") as ps_s,
            tc.tile_pool(name="ps_at", bufs=2, space="PSUM") as ps_at,
        ):
            # ---- Q-projection critical path first: wq/xq interleaved so
            # the k-chunked first q-unit can start streaming ASAP ----
            qT = [res.tile([P, TQ], dt.bfloat16, tag=f"qT{hp}", name=f"qT{hp}")
                  for hp in range(NHP)]
            xq_cache = {}

            def load_xq(qsb):
                if qsb in xq_cache:
                    return xq_cache[qsb]
                xqc = []
                for k in range(NDM):
                    t = xp.tile([P, SB], dt.bfloat16, tag="xq", name="xq")
                    nc.sync.dma_start(
                        t[:], xqT.ap()[k * P:(k + 1) * P,
                                       qsb * SB:(qsb + 1) * SB])
                    xqc.append(t)
                xq_cache[qsb] = xqc
                return xqc

            wq_t = []
            xq0 = []
            for k in range(NDM):
                t = res.tile([P, HL * DQ], dt.bfloat16, tag=f"wq{k}", name=f"wq{k}")
                nc.sync.dma_start(t[:], wq_d.ap()[k * P:(k + 1) * P, :])
                wq_t.append(t)
                t = xp.tile([P, SB], dt.bfloat16, tag="xq", name="xq")
                nc.sync.dma_start(t[:], xqT.ap()[k * P:(k + 1) * P, 0:SB])
                xq0.append(t)
            xq_cache[0] = xq0
            bq_t = res.tile([P, NHP], dt.float32, tag="bq", name="bq_t")
            nc.sync.dma_start(bq_t[:], bq_d.ap()[:, :])

            xkv_cache = {}
            wk_t, wv_t = [], []
            xkv0 = []
            for k in range(NDM):
                t = res.tile([P, HL * DQ], dt.bfloat16, tag=f"wk{k}", name=f"wk{k}")
                nc.sync.dma_start(t[:], wk_d.ap()[k * P:(k + 1) * P, :])
                wk_t.append(t)
                t = res.tile([P, HL * DV], dt.bfloat16, tag=f"wv{k}", name=f"wv{k}")
                nc.sync.dma_start(t[:], wv_d.ap()[k * P:(k + 1) * P, :])
                wv_t.append(t)
                t = xp.tile([P, SB], dt.bfloat16, tag="xkv", name="xkv")
                nc.sync.dma_start(t[:], xkvT.ap()[k * P:(k + 1) * P, 0:SB])
                xkv0.append(t)
            xkv_cache[0] = xkv0
            bk_t = res.tile([P, NHP], dt.float32, tag="bk", name="bk_t")
            nc.sync.dma_start(bk_t[:], bk_d.ap()[:, :])

            def load_xkv(nj):
                if nj in xkv_cache:
                    return xkv_cache[nj]
                c = []
                for k in range(NDM):
                    t = xp.tile([P, SB], dt.bfloat16, tag="xkv", name="xkv")
                    nc.sync.dma_start(
                        t[:], xkvT.ap()[k * P:(k + 1) * P,
                                        nj * SB:(nj + 1) * SB])
                    c.append(t)
                xkv_cache[nj] = c
                return c

            padm_t = res.tile([P, NKT], dt.float32, tag="padm", name="padm_t")
            nc.sync.dma_start(padm_t[:], padm_d.ap()[:, :])
            msk_t = res.tile([P, 2 * P], dt.bfloat16, tag="msk", name="msk_t")
            nc.sync.dma_start(msk_t[:], msk_d.ap()[:, :])
            sel_t = res.tile([P, HL * 64], dt.bfloat16, tag="sel", name="sel_t")
            nc.sync.dma_start(sel_t[:], one_d.ap()[:, :])
            wo_t = []
            for hp in range(NHP):
                t = res.tile([P, DM], dt.bfloat16, tag=f"wo{hp}", name=f"wo{hp}")
                nc.sync.dma_start(t[:], wo_d.ap()[hp * P:(hp + 1) * P, :])
                wo_t.append(t)

            kT = [res.tile([P, TKV], dt.bfloat16, tag=f"kT{hp}", name=f"kT{hp}")
                  for hp in range(NHP)]
            v_t = [res.tile([P, HL * 65], dt.bfloat16, tag=f"v{vt}", name=f"v{vt}")
                   for vt in range(NKT)]

            # ---- micro-step filler queue ----
            steps = []

            def fill(n):
                for _ in range(n):
                    if not steps:
                        return
                    steps.pop(0)()

            def q_unit_steps(qsb, hp):
                st = {}
                def mm(k):
                    def f():
                        if k == 0:
                            st["ps"] = ps_proj.tile([P, SB], dt.float32,
                                                    tag="pp", name="pp")
                        nc.tensor.matmul(
                            st["ps"][:],
                            wq_t[k][:, hp * P:(hp + 1) * P],
                            load_xq(qsb)[k][:],
                            start=(k == 0), stop=(k == NDM - 1),
                        )
                    return f
                def epi():
                    with nc.allow_low_precision(reason="bf16 Q"):
                        nc.vector.tensor_scalar_add(
                            qT[hp][:, qsb * SB:(qsb + 1) * SB], st["ps"][:],
                            bq_t[:, hp:hp + 1],
                        )
                return [mm(k) for k in range(NDM)] + [epi]

            def k_unit_steps(nj, hp):
                st = {}
                def mm(k):
                    def f():
                        if k == 0:
                            st["ps"] = ps_proj.tile([P, SB], dt.float32,
                                                    tag="pp", name="pp")
                        nc.tensor.matmul(
                            st["ps"][:],
                            wk_t[k][:, hp * P:(hp + 1) * P],
                            load_xkv(nj)[k][:],
                            start=(k == 0), stop=(k == NDM - 1),
                        )
                    return f
                def epi():
                    with nc.allow_low_precision(reason="bf16 K"):
                        nc.vector.tensor_scalar_add(
                            kT[hp][:, nj * SB:(nj + 1) * SB], st["ps"][:],
                            bk_t[:, hp:hp + 1],
                        )
                return [mm(k) for k in range(NDM)] + [epi]

            def v_unit_steps(vt):
                st = {}
                def mm(k):
                    def f():
                        if k == 0:
                            st["ps"] = ps_proj.tile([P, SB], dt.float32,
                                                    tag="pp", name="pp")
                        nc.tensor.matmul(
                            st["ps"][:],
                            load_xkv(vt // 4)[k][:, (vt % 4) * P:(vt % 4 + 1) * P],
                            wv_t[k][:],
                            start=(k == 0), stop=(k == NDM - 1),
                        )
                    return f
                def epi():
                    vtile = v_t[vt]
                    v3 = vtile[:].rearrange("p (h d) -> p h d", d=65)
                    with nc.allow_low_precision(reason="bf16 V"):
                        # padding folded in: padded kv rows scale to 0
                        nc.vector.tensor_scalar_mul(
                            v3[:, :, 0:64],
                            st["ps"][:].rearrange("p (h d) -> p h d", d=64),
                            padm_t[:, vt:vt + 1],
                        )
                    nc.gpsimd.memset(v3[:, :, 64:65], 1.0)
                def epi2():
                    v3 = v_t[vt][:].rearrange("p (h d) -> p h d", d=65)
                    with nc.allow_low_precision(reason="bf16 ones-col pad"):
                        nc.vector.tensor_scalar_mul(
                            v3[:, :, 64:65], v3[:, :, 64:65],
                            padm_t[:, vt:vt + 1],
                        )
                return [mm(k) for k in range(NDM)] + [epi, epi2]

            def fin_steps(blk):
                qsb, at_tiles, an_tiles, rc = blk
                def mk(h):
                    def f():
                        hp, off = h // 2, (h % 2) * 64
                        cg = (h // 4) * SB
                        # broadcast head h's recip row across 64 partitions
                        # via PE: sel[:, h*64:(h+1)*64] is 1.0 on row 32*(h%4)
                        bc = ps_proj.tile([P, SB], dt.float32, tag="pp",
                                          name="pp")
                        nc.tensor.matmul(
                            bc[0:64, :], sel_t[:, h * 64:(h + 1) * 64],
                            rc[:, cg:cg + SB],
                            start=True, stop=True,
                        )
                        with nc.allow_low_precision(reason="bf16 attn"):
                            nc.vector.tensor_mul(
                                at_tiles[hp][off:off + 64, :],
                                an_tiles[h][:], bc[0:64, :],
                            )
                    return f
                return [mk(h) for h in range(HL)]

            def oproj_unit_steps(qsb, at_tiles, qt, col):
                st = {}
                def mm(hp):
                    def f():
                        if hp == 0:
                            st["ps"] = ps_proj.tile([P, SB], dt.float32,
                                                    tag="pp", name="pp")
                        nc.tensor.matmul(
                            st["ps"][:],
                            at_tiles[hp][:, qt * P:(qt + 1) * P],
                            wo_t[hp][:, col * SB:(col + 1) * SB],
                            start=(hp == 0), stop=(hp == NHP - 1),
                        )
                    return f
                def epi():
                    ost = osp.tile([P, SB], dt.float32, tag="ost", name="ost")
                    nc.vector.tensor_copy(ost[:], st["ps"][:])
                    r0 = qsb * SB + qt * P
                    nc.scalar.dma_start(
                        out_d.ap()[r0:r0 + P, col * SB:(col + 1) * SB], ost[:]
                    )
                return [mm(hp) for hp in range(NHP)] + [epi]

            # ---- main loop over q super-blocks ----
            prev_blk = None
            for qsb in range(NQSB):
                # producers for THIS block (left over in the queue) must
                # all be emitted before its attention
                fill(len(steps))
                if qsb == 0:
                    for hp in range(NHP):
                        for s in q_unit_steps(0, hp):
                            s()
                    for hp in range(NHP):
                        for s in k_unit_steps(0, hp):
                            s()
                    for vt in range(4):
                        for s in v_unit_steps(vt):
                            s()

                # build the filler queue for this round: finalize+oproj of
                # the previous block, then next block's projections. The
                # fin steps wait on the reciprocal emitted at the previous
                # round's end, so a couple of projection units go first to
                # keep the in-order PE queue from stalling on it.
                if prev_blk is not None:
                    fins = fin_steps(prev_blk)
                    pq, pat = prev_blk[0], prev_blk[1]
                    ou = [oproj_unit_steps(pq, pat, qt, col)
                          for qt in range(4) for col in range(2)]
                else:
                    fins, ou = [], []
                pu = []
                if qsb + 1 < NQSB:
                    pq1 = qsb + 1
                    for hp in range(NHP):
                        pu.append(q_unit_steps(pq1, hp))
                    for hp in range(NHP):
                        pu.append(k_unit_steps(pq1, hp))
                    for vt in range(4 * pq1, 4 * pq1 + 4):
                        pu.append(v_unit_steps(vt))
                    # prefetch next block's activations onto the DMA queue
                    load_xq(pq1)
                    load_xkv(pq1)
                # queue order: a couple of projection units buffer the fin
                # steps (which wait on the reciprocal halves finishing on
                # DVE), then all fins (oproj contracts over every head),
                # then alternating projection/oproj units
                for u in pu[:2]:
                    steps.extend(u)
                steps.extend(fins)
                rest = []
                oi, pi = 0, 2
                while oi < len(ou) or pi < len(pu):
                    if pi < len(pu):
                        rest.append(pu[pi]); pi += 1
                    if oi < len(ou):
                        rest.append(ou[oi]); oi += 1
                for u in rest:
                    steps.extend(u)

                kt_max = 4 * qsb + 4 if causal else NKT
                n_iters = NHP * kt_max
                rate = max(1, -(-len(steps) // n_iters))

                at_tiles = [
                    atp.tile([P, SB], dt.bfloat16, tag=f"attnT{hp}",
                             name=f"attnT{hp}")
                    for hp in range(NHP)
                ]
                # normalizer rows gathered at partitions {0,32,64,96} x 2
                # column groups (DVE partition offsets must be 32-aligned);
                # filler 1.0 keeps the reciprocal finite on unused rows
                sums = rcp.tile([P, 2 * SB], dt.float32, tag="sums", name="sums")
                nc.gpsimd.memset(sums[:], 1.0)
                rc = rcp.tile([P, 2 * SB], dt.bfloat16, tag="rc", name="rc")
                an_tiles = []
                for hp in range(NHP):
                    aps2 = [
                        ps_at.tile([65, SB], dt.float32, tag="at", name="at")
                        for _ in range(2)
                    ]

                    def emit_pv(kt, pt, c0):
                        for e in range(2):
                            h = 2 * hp + e
                            nc.tensor.matmul(
                                aps2[e][:, c0:SB],
                                v_t[kt][:, h * 65:h * 65 + 65],
                                pt[:, e * SB + c0:(e + 1) * SB],
                                start=(kt == 0), stop=(kt == kt_max - 1),
                            )

                    pv_pending = []
                    for kt in range(kt_max):
                        # diagonal blocks (j>=1): only q columns >= 128*j can
                        # be unmasked -> trim the left columns entirely
                        j = kt - 4 * qsb if causal else -1
                        c0 = 128 * j if j > 0 else 0
                        sps = ps_s.tile([P, 2 * SB], dt.float32, tag="s",
                                        name="s")
                        for e in range(2):
                            off = e * 64
                            # the pair's S matmuls target disjoint PE row
                            # groups (0-63 / 64-127) and run concurrently
                            nc.tensor.matmul(
                                sps[:, e * SB + c0:(e + 1) * SB],
                                kT[hp][off:off + 64, kt * P:(kt + 1) * P],
                                qT[hp][off:off + 64,
                                       qsb * SB + c0:(qsb + 1) * SB],
                                start=True, stop=True,
                            )
                        pt = ptp.tile([P, 2 * SB], dt.bfloat16, tag="pT",
                                      name="pT")
                        s3 = sps[:].rearrange("p (e q) -> p e q", e=2)
                        p3 = pt[:].rearrange("p (e q) -> p e q", e=2)
                        nc.scalar.activation(
                            p3[:, :, c0:SB], s3[:, :, c0:SB], AF.Exp,
                            scale=0.125,
                        )
                        if causal and j >= 0:
                            # triangle band: only cols [c0, c0+128) can mask;
                            # runs on the otherwise-idle GPSIMD engine
                            nc.gpsimd.tensor_mul(
                                p3[:, :, c0:c0 + P], p3[:, :, c0:c0 + P],
                                msk_t[:].rearrange("p (e c) -> p e c", e=2),
                            )
                        # PV runs two kv-tiles behind S so the PE never
                        # head-of-line-stalls on the exp's ACT latency
                        pv_pending.append((kt, pt, c0))
                        if len(pv_pending) > 2:
                            emit_pv(*pv_pending.pop(0))
                        fill(rate)
                    for args in pv_pending:
                        emit_pv(*args)
                    for e in range(2):
                        h = 2 * hp + e
                        # stage numerator + normalizer row to SBUF, free psum
                        an = anp.tile([64, SB], dt.bfloat16, tag=f"an{h}",
                                      name=f"an{h}")
                        nc.scalar.copy(an[:], aps2[e][0:64, :])
                        r, cg = 32 * (h % 4), (h // 4) * SB
                        nc.scalar.copy(
                            sums[r:r + 1, cg:cg + SB], aps2[e][64:65, :]
                        )
                        an_tiles.append(an)
                    if hp == 1 or hp == 3:
                        # heads 0-3 land in cols 0:SB, heads 4-7 in SB:2SB;
                        # reciprocal each half as soon as its sums are
                        # staged so next round's fin steps don't stall
                        cg = 0 if hp == 1 else SB
                        with nc.allow_low_precision(reason="bf16 recip"):
                            nc.vector.reciprocal(
                                rc[:, cg:cg + SB], sums[:, cg:cg + SB]
                            )
                prev_blk = (qsb, at_tiles, an_tiles, rc)
            fill(len(steps))
            for s in fin_steps(prev_blk):
                s()
            for qt in range(4):
                for col in range(2):
                    for s in oproj_unit_steps(prev_blk[0], prev_blk[1],
                                              qt, col):
                        s()
    return nc


def _get_program(causal: bool):
    key = bool(causal)
    if key not in _programs:
        _programs[key] = build_program(key)
    return _programs[key]


def kernel(**inputs):
    from concourse.bass_utils import run_bass_kernel_spmd

    xq = np.asarray(inputs["query_sequence"], dtype=np.float32)
    xkv = np.asarray(inputs["key_value_sequence"], dtype=np.float32)
    pmask = np.asarray(inputs["key_value_padding_mask"])
    Wq = np.asarray(inputs["Wq"], dtype=np.float32)
    bq = np.asarray(inputs["bq"], dtype=np.float32)
    Wkv = np.asarray(inputs["Wkv"], dtype=np.float32)
    bkv = np.asarray(inputs["bkv"], dtype=np.float32)
    Wo = np.asarray(inputs["Wo"], dtype=np.float32)
    bo = np.asarray(inputs["bo"], dtype=np.float32)
    causal = bool(np.asarray(inputs["apply_causal_mask"]))

    nc = _get_program(causal)

    Wk_full = Wkv[:, : H * DQ]
    Wv_full = Wkv[:, H * DQ:]
    bk_full = bkv[: H * DQ]
    bv_full = bkv[H * DQ:]

    # causal triangle band mask, duplicated for the e-batched [P, 2, P]
    # layout: tri[kv, q] = 1 if q >= kv within a 128x128 diagonal block
    kvi = np.arange(P)[:, None]
    qi = np.arange(P)[None, :]
    tri = (qi >= kvi).astype(np.float32)
    msk2 = np.concatenate([tri, tri], axis=1).astype(bf16)
    # head-row selector: sel[:, h*64:(h+1)*64] = 1.0 on row 32*(h%4) else 0
    sel = np.zeros((P, HL * 64), np.float32)
    for h in range(HL):
        sel[32 * (h % 4), h * 64:(h + 1) * 64] = 1.0
    sel = sel.astype(bf16)

    in_maps = []
    for c in range(NC):
        b, g = divmod(c, 2)
        hs = slice(g * HL * DQ, (g + 1) * HL * DQ)
        padm = np.where(pmask[b], np.float32(0.0), np.float32(1.0))
        in_maps.append({
            "xqT": np.ascontiguousarray(xq[b].T).astype(bf16),
            "xkvT": np.ascontiguousarray(xkv[b].T).astype(bf16),
            "wq": np.ascontiguousarray(Wq[:, hs]).astype(bf16),
            "wk": np.ascontiguousarray(Wk_full[:, hs]).astype(bf16),
            "wv": np.ascontiguousarray(Wv_full[:, hs]).astype(bf16),
            "wo": np.ascontiguousarray(Wo[hs, :]).astype(bf16),
            "bqp": np.ascontiguousarray(bq[hs].reshape(NHP, P).T),
            "bkp": np.ascontiguousarray(bk_full[hs].reshape(NHP, P).T),
            "padm": np.ascontiguousarray(padm.reshape(NKT, P).T),
            "msk2": msk2,
            "one64": sel,
        })

    global _last_in_maps
    _last_in_maps = in_maps
    res = run_bass_kernel_spmd(nc, in_maps, core_ids=list(range(NC)))

    host_bias = bo + bv_full @ Wo  # softmax rows sum to 1 -> V-bias is additive
    out = np.empty((B, TQ, DM), np.float32)
    for b in range(B):
        out[b] = res.results[2 * b]["out"] + res.results[2 * b + 1]["out"] + host_bias
    return out


# revision 25
# speedup vs baseline: 1.0629x; 1.0629x over previous
# Multi-head attention (B=4, T=2048, D=1024, H=16, dqk=dv=64) on 8 trn2
# NeuronCores. Sharding: core c -> batch c//2, head-group c%2 (8 heads).
# Each core computes its batch's Q^T/K^T/V projections for its heads,
# causal flash attention with transposed scores (S^T[kv,q]; softmax
# normalizer via a ones-column appended to V), and a partial output
# projection. Host sums the two partials per batch and adds biases.
#
# Per-core structure: the attention inner loop is ACT(exp)-limited, so
# exp is batched across the head pair (one [128,2,512] ACTIVATE per kv
# tile), the padding mask is folded multiplicatively into V (uniform
# zero ACT bias), the causal mask multiply covers only the 128-col
# triangle band, the softmax-normalizer broadcast runs on the idle
# GPSIMD instead of the PE, and all non-attention matmul work
# (projections, output projection, finalize) is drained from a global
# micro-step queue at a fixed rate per attention iteration so the PE
# stream stays dense (HAM stays warm).
import numpy as np
import ml_dtypes

B, TQ, TKV, DM, H, DQ, DV = 4, 2048, 2048, 1024, 16, 64, 64
NC = 8          # cores
HL = 8          # heads per core
NHP = HL // 2   # 128-partition head-pair tiles (4)
SB = 512        # q super-block width
NQSB = TQ // SB
NKT = TKV // 128
NDM = DM // 128
P = 128

bf16 = ml_dtypes.bfloat16

_programs = {}
_last_in_maps = None


def _make_tc_class(tile_mod):
    from concourse.vector_clock import ScopedClock
    import concourse.mybir as mybir

    class TC(tile_mod.TileContext):
        # This toolchain's walrus codegen encodes at most ONE sync wait
        # per instruction. Tile's wait assignment can attach several, so
        # before lowering, peel extra waits off onto standalone
        # InstEventSemaphore instructions placed immediately before the
        # instruction on the same engine (in-order execution makes this
        # semantically identical).
        def _lower_ordered_insts(self, ordered):
            for bb_name, insts in ordered.items():
                out = []
                for inst in insts:
                    si = getattr(inst, "sync_info", None)
                    eng = getattr(inst, "engine", None)
                    if (
                        si is not None
                        and si.on_wait
                        and len(si.on_wait) > 1
                        and eng is not None
                        and eng != mybir.EngineType.Unassigned
                    ):
                        waits = list(si.on_wait)
                        for w in waits[:-1]:
                            ev = mybir.InstEventSemaphore(
                                name=f"I-{self.nc.next_id()}", ins=[], outs=[]
                            )
                            ev.engine = eng
                            ev.sync_info = mybir.SyncInfo(
                                on_wait=[w], on_update=[]
                            )
                            out.append(ev)
                        si.on_wait = waits[-1:]
                    out.append(inst)
                insts[:] = out
            return super()._lower_ordered_insts(ordered)

        # Same 1-wait limit applies to the tail drain; split its waits
        # into standalone wait instructions.
        def _drain_and_barrier(self, tick_clock, wait_clock):
            drain_inst = self.nc.sync.drain()
            wait_clock.add_sem_waits(
                drain_inst.ins, ScopedClock({None: tick_clock.global_clock})
            )
            si = drain_inst.ins.sync_info
            waits = list(si.on_wait) if si and si.on_wait else []
            if len(waits) > 1:
                si.on_wait = waits[:1]
                name2sem = {}
                for s in self.sems.allocated().values():
                    name2sem[getattr(s, "name", None) or str(s)] = s
                for w in waits[1:]:
                    self.nc.sync.wait_ge(name2sem[w.ant_name], w.wait_value)
            self.nc.all_engine_barrier()
            popped = self.nc._tile_sem_poison_stack.pop()
            assert popped is self._sem_poison
            self.nc.clear_and_free_semaphores(list(self.sems.allocated().values()))
            self.nc.all_engine_barrier()

    return TC


def build_program(causal: bool):
    import concourse.bass as bass
    import concourse.mybir as mybir
    import concourse.tile as tile

    dt = mybir.dt
    AF = mybir.ActivationFunctionType
    TC = _make_tc_class(tile)

    nc = bass.Bass("TRN2", target_bir_lowering=False, debug=False, num_devices=NC)

    xqT = nc.dram_tensor("xqT", [DM, TQ], dt.bfloat16, kind="ExternalInput")
    xkvT = nc.dram_tensor("xkvT", [DM, TKV], dt.bfloat16, kind="ExternalInput")
    wq_d = nc.dram_tensor("wq", [DM, HL * DQ], dt.bfloat16, kind="ExternalInput")
    wk_d = nc.dram_tensor("wk", [DM, HL * DQ], dt.bfloat16, kind="ExternalInput")
    wv_d = nc.dram_tensor("wv", [DM, HL * DV], dt.bfloat16, kind="ExternalInput")
    wo_d = nc.dram_tensor("wo", [HL * DV, DM], dt.bfloat16, kind="ExternalInput")
    bq_d = nc.dram_tensor("bqp", [P, NHP], dt.float32, kind="ExternalInput")
    bk_d = nc.dram_tensor("bkp", [P, NHP], dt.float32, kind="ExternalInput")
    padm_d = nc.dram_tensor("padm", [P, NKT], dt.float32, kind="ExternalInput")
    msk_d = nc.dram_tensor("msk2", [P, 2 * P], dt.bfloat16, kind="ExternalInput")
    one_d = nc.dram_tensor("one64", [P, HL * 64], dt.bfloat16, kind="ExternalInput")
    out_d = nc.dram_tensor("out", [TQ, DM], dt.float32, kind="ExternalOutput")

    with TC(nc) as tc:
        with (
            tc.tile_pool(name="res", bufs=1) as res,
            tc.tile_pool(name="xp", bufs=16) as xp,
            tc.tile_pool(name="ptp", bufs=4) as ptp,
            tc.tile_pool(name="atp", bufs=2) as atp,
            tc.tile_pool(name="anp", bufs=2) as anp,
            tc.tile_pool(name="rcp", bufs=2) as rcp,
            tc.tile_pool(name="osp", bufs=3) as osp,
            tc.tile_pool(name="ps_proj", bufs=2, space="PSUM") as ps_proj,
            tc.tile_pool(name="ps_s", bufs=2, space="PSUM") as ps_s,
            tc.tile_pool(name="ps_at", bufs=2, space="PSUM") as ps_at,
        ):
            # ---- Q-projection critical path first: wq/xq interleaved so
            # the k-chunked first q-unit can start streaming ASAP ----
            qT = [res.tile([P, TQ], dt.bfloat16, tag=f"qT{hp}", name=f"qT{hp}")
                  for hp in range(NHP)]
            xq_cache = {}

            def load_xq(qsb):
                if qsb in xq_cache:
                    return xq_cache[qsb]
                xqc = []
                for k in range(NDM):
                    t = xp.tile([P, SB], dt.bfloat16, tag="xq", name="xq")
                    nc.sync.dma_start(
                        t[:], xqT.ap()[k * P:(k + 1) * P,
                                       qsb * SB:(qsb + 1) * SB])
                    xqc.append(t)
                xq_cache[qsb] = xqc
                return xqc

            wq_t = []
            xq0 = []
            for k in range(NDM):
                t = res.tile([P, HL * DQ], dt.bfloat16, tag=f"wq{k}", name=f"wq{k}")
                nc.sync.dma_start(t[:], wq_d.ap()[k * P:(k + 1) * P, :])
                wq_t.append(t)
                t = xp.tile([P, SB], dt.bfloat16, tag="xq", name="xq")
                nc.sync.dma_start(t[:], xqT.ap()[k * P:(k + 1) * P, 0:SB])
                xq0.append(t)
            xq_cache[0] = xq0
            bq_t = res.tile([P, NHP], dt.float32, tag="bq", name="bq_t")
            nc.sync.dma_start(bq_t[:], bq_d.ap()[:, :])

            xkv_cache = {}
            wk_t, wv_t = [], []
            xkv0 = []
            for k in range(NDM):
                t = res.tile([P, HL * DQ], dt.bfloat16, tag=f"wk{k}", name=f"wk{k}")
                nc.sync.dma_start(t[:], wk_d.ap()[k * P:(k + 1) * P, :])
                wk_t.append(t)
                t = res.tile([P, HL * DV], dt.bfloat16, tag=f"wv{k}", name=f"wv{k}")
                nc.sync.dma_start(t[:], wv_d.ap()[k * P:(k + 1) * P, :])
                wv_t.append(t)
                t = xp.tile([P, SB], dt.bfloat16, tag="xkv", name="xkv")
                nc.sync.dma_start(t[:], xkvT.ap()[k * P:(k + 1) * P, 0:SB])
                xkv0.append(t)
            xkv_cache[0] = xkv0
            bk_t = res.tile([P, NHP], dt.float32, tag="bk", name="bk_t")
            nc.sync.dma_start(bk_t[:], bk_d.ap()[:, :])

            def load_xkv(nj):
                if nj in xkv_cache:
                    return xkv_cache[nj]
                c = []
                for k in range(NDM):
                    t = xp.tile([P, SB], dt.bfloat16, tag="xkv", name="xkv")
                    nc.sync.dma_start(
                        t[:], xkvT.ap()[k * P:(k + 1) * P,
                                        nj * SB:(nj + 1) * SB])
                    c.append(t)
                xkv_cache[nj] = c
                return c

            padm_t = res.tile([P, NKT], dt.float32, tag="padm", name="padm_t")
            nc.sync.dma_start(padm_t[:], padm_d.ap()[:, :])
            msk_t = res.tile([P, 2 * P], dt.bfloat16, tag="msk", name="msk_t")
            nc.sync.dma_start(msk_t[:], msk_d.ap()[:, :])
            sel_t = res.tile([P, HL * 64], dt.bfloat16, tag="sel", name="sel_t")
            nc.sync.dma_start(sel_t[:], one_d.ap()[:, :])
            wo_t = []
            for hp in range(NHP):
                t = res.tile([P, DM], dt.bfloat16, tag=f"wo{hp}", name=f"wo{hp}")
                nc.sync.dma_start(t[:], wo_d.ap()[hp * P:(hp + 1) * P, :])
                wo_t.append(t)

            kT = [res.tile([P, TKV], dt.bfloat16, tag=f"kT{hp}", name=f"kT{hp}")
                  for hp in range(NHP)]
            v_t = [res.tile([P, HL * 65], dt.bfloat16, tag=f"v{vt}", name=f"v{vt}")
                   for vt in range(NKT)]

            # ---- micro-step filler queue ----
            steps = []

            def fill(n):
                for _ in range(n):
                    if not steps:
                        return
                    steps.pop(0)()

            def q_unit_steps(qsb, hp):
                st = {}
                def mm(k):
                    def f():
                        if k == 0:
                            st["ps"] = ps_proj.tile([P, SB], dt.float32,
                                                    tag="pp", name="pp")
                        nc.tensor.matmul(
                            st["ps"][:],
                            wq_t[k][:, hp * P:(hp + 1) * P],
                            load_xq(qsb)[k][:],
                            start=(k == 0), stop=(k == NDM - 1),
                        )
                    return f
                def epi():
                    with nc.allow_low_precision(reason="bf16 Q"):
                        nc.vector.tensor_scalar_add(
                            qT[hp][:, qsb * SB:(qsb + 1) * SB], st["ps"][:],
                            bq_t[:, hp:hp + 1],
                        )
                return [mm(k) for k in range(NDM)] + [epi]

            def k_unit_steps(nj, hp):
                st = {}
                def mm(k):
                    def f():
                        if k == 0:
                            st["ps"] = ps_proj.tile([P, SB], dt.float32,
                                                    tag="pp", name="pp")
                        nc.tensor.matmul(
                            st["ps"][:],
                            wk_t[k][:, hp * P:(hp + 1) * P],
                            load_xkv(nj)[k][:],
                            start=(k == 0), stop=(k == NDM - 1),
                        )
                    return f
                def epi():
                    with nc.allow_low_precision(reason="bf16 K"):
                        nc.vector.tensor_scalar_add(
                            kT[hp][:, nj * SB:(nj + 1) * SB], st["ps"][:],
                            bk_t[:, hp:hp + 1],
                        )
                return [mm(k) for k in range(NDM)] + [epi]

            def v_unit_steps(vt):
                st = {}
                def mm(k):
                    def f():
                        if k == 0:
                            st["ps"] = ps_proj.tile([P, SB], dt.float32,
                                                    tag="pp", name="pp")
                        nc.tensor.matmul(
                            st["ps"][:],
                            load_xkv(vt // 4)[k][:, (vt % 4) * P:(vt % 4 + 1) * P],
                            wv_t[k][:],
                            start=(k == 0), stop=(k == NDM - 1),
                        )
                    return f
                def epi():
                    vtile = v_t[vt]
                    v3 = vtile[:].rearrange("p (h d) -> p h d", d=65)
                    with nc.allow_low_precision(reason="bf16 V"):
                        # padding folded in: padded kv rows scale to 0
                        nc.vector.tensor_scalar_mul(
                            v3[:, :, 0:64],
                            st["ps"][:].rearrange("p (h d) -> p h d", d=64),
                            padm_t[:, vt:vt + 1],
                        )
                    nc.gpsimd.memset(v3[:, :, 64:65], 1.0)
                def epi2():
                    v3 = v_t[vt][:].rearrange("p (h d) -> p h d", d=65)
                    with nc.allow_low_precision(reason="bf16 ones-col pad"):
                        nc.vector.tensor_scalar_mul(
                            v3[:, :, 64:65], v3[:, :, 64:65],
                            padm_t[:, vt:vt + 1],
                        )
                return [mm(k) for k in range(NDM)] + [epi, epi2]

            def fin_steps(blk):
                qsb, at_tiles, an_tiles, rc = blk
                def mk(h):
                    def f():
                        hp, off = h // 2, (h % 2) * 64
                        cg = (h // 4) * SB
                        # broadcast head h's recip row across 64 partitions
                        # via PE: sel[:, h*64:(h+1)*64] is 1.0 on row 32*(h%4)
                        bc = ps_proj.tile([P, SB], dt.float32, tag="pp",
                                          name="pp")
                        nc.tensor.matmul(
                            bc[0:64, :], sel_t[:, h * 64:(h + 1) * 64],
                            rc[:, cg:cg + SB],
                            start=True, stop=True,
                        )
                        with nc.allow_low_precision(reason="bf16 attn"):
                            nc.vector.tensor_mul(
                                at_tiles[hp][off:off + 64, :],
                                an_tiles[h][:], bc[0:64, :],
                            )
                    return f
                return [mk(h) for h in range(HL)]

            def oproj_unit_steps(qsb, at_tiles, qt, col):
                st = {}
                def mm(hp):
                    def f():
                        if hp == 0:
                            st["ps"] = ps_proj.tile([P, SB], dt.float32,
                                                    tag="pp", name="pp")
                        nc.tensor.matmul(
                            st["ps"][:],
                            at_tiles[hp][:, qt * P:(qt + 1) * P],
                            wo_t[hp][:, col * SB:(col + 1) * SB],
                            start=(hp == 0), stop=(hp == NHP - 1),
                        )
                    return f
                def epi():
                    ost = osp.tile([P, SB], dt.float32, tag="ost", name="ost")
                    nc.vector.tensor_copy(ost[:], st["ps"][:])
                    r0 = qsb * SB + qt * P
                    nc.sync.dma_start(
                        out_d.ap()[r0:r0 + P, col * SB:(col + 1) * SB], ost[:]
                    )
                return [mm(hp) for hp in range(NHP)] + [epi]

            # ---- main loop over q super-blocks ----
            # q-blocks are processed out of order: exp (ACT) work scales
            # with the causal extent, so exp-heavy late blocks pair with
            # rounds that still have projection filler for the PE. kv
            # block j is produced in the round before its first use.
            sched = [0, 2, 3, 1] if causal else list(range(NQSB))
            kv_done = {0}
            prev_blk = None
            for idx, qsb in enumerate(sched):
                # producers for THIS block (left over in the queue) must
                # all be emitted before its attention
                fill(len(steps))
                if idx == 0:
                    for hp in range(NHP):
                        for s in q_unit_steps(qsb, hp):
                            s()
                    for nj in ([0] if causal else list(range(NKT // 4))):
                        for hp in range(NHP):
                            for s in k_unit_steps(nj, hp):
                                s()
                        for vt in range(4 * nj, 4 * nj + 4):
                            for s in v_unit_steps(vt):
                                s()
                        kv_done.add(nj)

                # build the filler queue for this round: finalize+oproj of
                # the previous block, then next block's projections. The
                # fin steps wait on the reciprocal emitted at the previous
                # round's end, so a couple of projection units go first to
                # keep the in-order PE queue from stalling on it.
                if prev_blk is not None:
                    fins = fin_steps(prev_blk)
                    pq, pat = prev_blk[0], prev_blk[1]
                    ou = [oproj_unit_steps(pq, pat, qt, col)
                          for qt in range(4) for col in range(2)]
                else:
                    fins, ou = [], []
                pu = []
                if idx + 1 < len(sched):
                    pq1 = sched[idx + 1]
                    for hp in range(NHP):
                        pu.append(q_unit_steps(pq1, hp))
                    load_xq(pq1)
                    for nj in range(pq1 + 1):
                        if nj in kv_done:
                            continue
                        kv_done.add(nj)
                        load_xkv(nj)
                        for hp in range(NHP):
                            pu.append(k_unit_steps(nj, hp))
                        for vt in range(4 * nj, 4 * nj + 4):
                            pu.append(v_unit_steps(vt))
                # queue order: a couple of projection units buffer the fin
                # steps (which wait on the reciprocal halves finishing on
                # DVE), then all fins (oproj contracts over every head),
                # then alternating projection/oproj units
                for u in pu[:2]:
                    steps.extend(u)
                steps.extend(fins)
                rest = []
                oi, pi = 0, 2
                while oi < len(ou) or pi < len(pu):
                    if pi < len(pu):
                        rest.append(pu[pi]); pi += 1
                    if oi < len(ou):
                        rest.append(ou[oi]); oi += 1
                for u in rest:
                    steps.extend(u)

                kt_max = 4 * qsb + 4 if causal else NKT
                n_iters = NHP * kt_max
                rate = max(1, -(-len(steps) // n_iters))

                at_tiles = [
                    atp.tile([P, SB], dt.bfloat16, tag=f"attnT{hp}",
                             name=f"attnT{hp}")
                    for hp in range(NHP)
                ]
                # normalizer rows gathered at partitions {0,32,64,96} x 2
                # column groups (DVE partition offsets must be 32-aligned);
                # filler 1.0 keeps the reciprocal finite on unused rows
                sums = rcp.tile([P, 2 * SB], dt.float32, tag="sums", name="sums")
                nc.gpsimd.memset(sums[:], 1.0)
                rc = rcp.tile([P, 2 * SB], dt.bfloat16, tag="rc", name="rc")
                an_tiles = []
                for hp in range(NHP):
                    aps2 = [
                        ps_at.tile([65, SB], dt.float32, tag="at", name="at")
                        for _ in range(2)
                    ]

                    def emit_pv(kt, pt, c0):
                        for e in range(2):
                            h = 2 * hp + e
                            nc.tensor.matmul(
                                aps2[e][:, c0:SB],
                                v_t[kt][:, h * 65:h * 65 + 65],
                                pt[:, e * SB + c0:(e + 1) * SB],
                                start=(kt == 0), stop=(kt == kt_max - 1),
                            )

                    pv_pending = []
                    for kt in range(kt_max):
                        # diagonal blocks (j>=1): only q columns >= 128*j can
                        # be unmasked -> trim the left columns entirely
                        j = kt - 4 * qsb if causal else -1
                        c0 = 128 * j if j > 0 else 0
                        sps = ps_s.tile([P, 2 * SB], dt.float32, tag="s",
                                        name="s")
                        for e in range(2):
                            off = e * 64
                            # the pair's S matmuls target disjoint PE row
                            # groups (0-63 / 64-127) and run concurrently
                            nc.tensor.matmul(
                                sps[:, e * SB + c0:(e + 1) * SB],
                                kT[hp][off:off + 64, kt * P:(kt + 1) * P],
                                qT[hp][off:off + 64,
                                       qsb * SB + c0:(qsb + 1) * SB],
                                start=True, stop=True,
                            )
                        pt = ptp.tile([P, 2 * SB], dt.bfloat16, tag="pT",
                                      name="pT")
                        s3 = sps[:].rearrange("p (e q) -> p e q", e=2)
                        p3 = pt[:].rearrange("p (e q) -> p e q", e=2)
                        nc.scalar.activation(
                            p3[:, :, c0:SB], s3[:, :, c0:SB], AF.Exp,
                            scale=0.125,
                        )
                        if causal and j >= 0:
                            # triangle band: only cols [c0, c0+128) can mask;
                            # runs on the otherwise-idle GPSIMD engine
                            nc.gpsimd.tensor_mul(
                                p3[:, :, c0:c0 + P], p3[:, :, c0:c0 + P],
                                msk_t[:].rearrange("p (e c) -> p e c", e=2),
                            )
                        # PV runs two kv-tiles behind S so the PE never
                        # head-of-line-stalls on the exp's ACT latency
                        pv_pending.append((kt, pt, c0))
                        if len(pv_pending) > 2:
                            emit_pv(*pv_pending.pop(0))
                        fill(rate)
                    for args in pv_pending:
                        emit_pv(*args)
                    for e in range(2):
                        h = 2 * hp + e
                        # stage numerator + normalizer row to SBUF, free psum
                        an = anp.tile([64, SB], dt.bfloat16, tag=f"an{h}",
                                      name=f"an{h}")
                        nc.vector.tensor_copy(an[:], aps2[e][0:64, :])
                        r, cg = 32 * (h % 4), (h // 4) * SB
                        nc.vector.tensor_copy(
                            sums[r:r + 1, cg:cg + SB], aps2[e][64:65, :]
                        )
                        an_tiles.append(an)
                    if hp == 1 or hp == 3:
                        # heads 0-3 land in cols 0:SB, heads 4-7 in SB:2SB;
                        # reciprocal each half as soon as its sums are
                        # staged so next round's fin steps don't stall
                        cg = 0 if hp == 1 else SB
                        with nc.allow_low_precision(reason="bf16 recip"):
                            nc.vector.reciprocal(
                                rc[:, cg:cg + SB], sums[:, cg:cg + SB]
                            )
                prev_blk = (qsb, at_tiles, an_tiles, rc)
            fill(len(steps))
            for s in fin_steps(prev_blk):
                s()
            for qt in range(4):
                for col in range(2):
                    for s in oproj_unit_steps(prev_blk[0], prev_blk[1],
                                              qt, col):
                        s()
    return nc


def _get_program(causal: bool):
    key = bool(causal)
    if key not in _programs:
        _programs[key] = build_program(key)
    return _programs[key]


def kernel(**inputs):
    from concourse.bass_utils import run_bass_kernel_spmd

    xq = np.asarray(inputs["query_sequence"], dtype=np.float32)
    xkv = np.asarray(inputs["key_value_sequence"], dtype=np.float32)
    pmask = np.asarray(inputs["key_value_padding_mask"])
    Wq = np.asarray(inputs["Wq"], dtype=np.float32)
    bq = np.asarray(inputs["bq"], dtype=np.float32)
    Wkv = np.asarray(inputs["Wkv"], dtype=np.float32)
    bkv = np.asarray(inputs["bkv"], dtype=np.float32)
    Wo = np.asarray(inputs["Wo"], dtype=np.float32)
    bo = np.asarray(inputs["bo"], dtype=np.float32)
    causal = bool(np.asarray(inputs["apply_causal_mask"]))

    nc = _get_program(causal)

    Wk_full = Wkv[:, : H * DQ]
    Wv_full = Wkv[:, H * DQ:]
    bk_full = bkv[: H * DQ]
    bv_full = bkv[H * DQ:]

    # causal triangle band mask, duplicated for the e-batched [P, 2, P]
    # layout: tri[kv, q] = 1 if q >= kv within a 128x128 diagonal block
    kvi = np.arange(P)[:, None]
    qi = np.arange(P)[None, :]
    tri = (qi >= kvi).astype(np.float32)
    msk2 = np.concatenate([tri, tri], axis=1).astype(bf16)
    # head-row selector: sel[:, h*64:(h+1)*64] = 1.0 on row 32*(h%4) else 0
    sel = np.zeros((P, HL * 64), np.float32)
    for h in range(HL):
        sel[32 * (h % 4), h * 64:(h + 1) * 64] = 1.0
    sel = sel.astype(bf16)

    in_maps = []
    for c in range(NC):
        b, g = divmod(c, 2)
        hs = slice(g * HL * DQ, (g + 1) * HL * DQ)
        padm = np.where(pmask[b], np.float32(0.0), np.float32(1.0))
        in_maps.append({
            "xqT": np.ascontiguousarray(xq[b].T).astype(bf16),
            "xkvT": np.ascontiguousarray(xkv[b].T).astype(bf16),
            "wq": np.ascontiguousarray(Wq[:, hs]).astype(bf16),
            "wk": np.ascontiguousarray(Wk_full[:, hs]).astype(bf16),
            "wv": np.ascontiguousarray(Wv_full[:, hs]).astype(bf16),
            "wo": np.ascontiguousarray(Wo[hs, :]).astype(bf16),
            "bqp": np.ascontiguousarray(bq[hs].reshape(NHP, P).T),
            "bkp": np.ascontiguousarray(bk_full[hs].reshape(NHP, P).T),
            "padm": np.ascontiguousarray(padm.reshape(NKT, P).T),
            "msk2": msk2,
            "one64": sel,
        })

    global _last_in_maps
    _last_in_maps = in_maps
    res = run_bass_kernel_spmd(nc, in_maps, core_ids=list(range(NC)))

    host_bias = bo + bv_full @ Wo  # softmax rows sum to 1 -> V-bias is additive
    out = np.empty((B, TQ, DM), np.float32)
    for b in range(B):
        out[b] = res.results[2 * b]["out"] + res.results[2 * b + 1]["out"] + host_bias
    return out
